# revision 46
# baseline (speedup 1.0000x reference)
"""Trainium2 Bass kernel for nn_Colorizer (retrieval_knn).

Computation (per reference frame r of 3, for each pixel p of a 128x128 image):
  corr[r, n, p] = <feats_t[:, p], feats_r[r, :, p + offset(n)]>   n in 13x13 window
  q_val[r, p]  = max_n corr ; q_idx[r, p] = argmax_n corr (first occurrence)
  gathered[r, c, p] = quantized_sub[r, c, p + offset(q_idx)]      (zero padded)
  out[c, p] = sum_r softmax_r(q_val)[r] * gathered[r, c, p]

Sharding: the spatial h dim is split into 8 bands of 16 rows (one per core);
each core handles all 3 refs for its band, so the softmax over refs is local
and no device collective is needed.  Host reassembles the row bands.

Device algorithm per core, per (tile, ref) pair (16 tiles of 16x8 pixels,
3 refs -> 48 pairs):
  * TensorE computes the Gram matrix between the tile's feats_t vectors and
    the 28x20 zero-padded feats_r halo window (560 columns over 2 PSUM banks)
    as a 3-pass fp16 split (hi*hi + hi*lo + lo*hi; the dropped lo*lo term is
    ~6e-6 which is far below the 3.3e-5 min top-2 corr gap on these inputs,
    so argmax and softmax match fp32), then accumulates a -1e30 valid-window
    mask via a bf16 identity matmul.  A dozen dummy matmuls on the
    already-loaded mask warm the PE's HAM clock gate during input load.
  * ScalarE (ACT) copies each masked PSUM pair to a contiguous SBUF stream,
    releasing PSUM for the next matmuls.
  * VectorE computes per-pair maxes (tensor_reduce, batched per find group)
    and argmaxes (one find_index8 per group: the group's pair maxes are the
    8 match slots, the group's concatenated corr segments are the stream, so
    the found stream position is directly the gather-table row).  Group
    sizes ramp 1,1,2,4,8... so the serial gpsimd gather stream starts as
    early as possible.
  * GpSimd issues one indirect DMA per pair ([128,1] offsets - the SWDGE
    only honors one offset per partition) gathering the argmax pixel
    (3 channels) from the group's DRAM table.
  * A tiny fp32 softmax over the 3 refs weights the gathered values.

Engine budget per core (measured): PE ~50us (warm ~283ns/280-col matmul),
DVE ~63us (the 2 scans/pair at ~1 elem/cycle are the floor), ScalarE ~36us,
GpSimd gathers 48 x ~1.4us (descriptor generation on the Q7 cores; this
serial stream plus the DVE stream set the ~100us span), ~14us of fixed
NEFF preamble/exit-drain overhead.
"""

import os

import numpy as np

import concourse.bass as bass
import concourse.mybir as mybir
import concourse.tile as tile
from concourse import bacc
from concourse.bass import IndirectOffsetOnAxis
from concourse.bass_utils import run_bass_kernel_spmd

F32 = mybir.dt.float32
F16 = mybir.dt.float16
BF16 = mybir.dt.bfloat16
U32 = mybir.dt.uint32

NCORES = 8
NREF, C, H, W = 3, 128, 128, 128
RAD = 6                      # patch radius
PS = 2 * RAD + 1             # 13
CQ = 3                       # quantized channels
SUB = 4                      # quantized_r spatial subsample stride

ROWS = H // NCORES           # 16 rows per core
XB = 8                       # x block size
NT = W // XB                 # 16 tiles per ref
WY = ROWS + 2 * RAD          # 28 window rows
WX = XB + 2 * RAD            # 20 window cols
WIN = WY * WX                # 560
HALF = WY // 2               # 14 window rows per PSUM bank
NHALF = HALF * WX            # 280 columns per matmul
PW = W + 2 * RAD             # 140 padded width
XA = 76                      # x-block A: padded cols [0, 76), tiles 0-7
XB2 = 84                     # x-block B: padded cols [56, 140), tiles 8-15
XB0 = PW - XB2               # 56, B block origin
NP = NREF * NT               # 48 (tile, ref) pairs
NB = 6                       # corr-buffer batches
BP = NP // NB                # 8 pairs per batch
# find groups: small at the start (begin the serial gpsimd gather stream
# early) and at the end (shorten the post-last-find gather tail),
# 8-wide in the middle (less DVE overhead)
_GRP_SIZES = [1, 1, 2, 4, 8, 8, 8, 8, 8]
FIND_GROUPS = []
_s = 0
for _n in _GRP_SIZES:
    FIND_GROUPS.append(list(range(_s, _s + _n)))
    _s += _n
NEG = -1.0e30

_CACHE: dict = {}


def _build_program(debug_taps: bool = False) -> bacc.Bacc:
    nc = bacc.Bacc("TRN2", target_bir_lowering=False, debug=False)

    fth_d = nc.dram_tensor("fth", [C, NT * 128], F16, kind="ExternalInput")
    ftl_d = nc.dram_tensor("ftl", [C, NT * 128], F16, kind="ExternalInput")
    # fr windows pre-split into two overlapping contiguous x-blocks:
    # A = padded cols [0, XA) (tiles 0-7), B = [PW-XB2, PW) (tiles 8-15)
    frhA_d = nc.dram_tensor("frhA", [NREF, C, WY * XA], F16, kind="ExternalInput")
    frlA_d = nc.dram_tensor("frlA", [NREF, C, WY * XA], F16, kind="ExternalInput")
    frhB_d = nc.dram_tensor("frhB", [NREF, C, WY * XB2], F16, kind="ExternalInput")
    frlB_d = nc.dram_tensor("frlB", [NREF, C, WY * XB2], F16, kind="ExternalInput")
    mask_d = nc.dram_tensor("mask", [128, WIN], BF16, kind="ExternalInput")
    ident_d = nc.dram_tensor("ident", [128, 128], BF16, kind="ExternalInput")
    # per-find-group gather tables: row j*WIN + n = quantized pixel (3ch)
    # of window position n for the group's j-th pair (find_index8 returns
    # stream positions, which are then direct row indices)
    qrp_d = [
        nc.dram_tensor(f"qrp{g}", [len(grp) * WIN, CQ], F32, kind="ExternalInput")
        for g, grp in enumerate(FIND_GROUPS)
    ]
    # raw layout [pixel_partition=(yl,xl), tile, channel]; host untangles
    out_d = nc.dram_tensor("out", [128, NT * CQ], F32, kind="ExternalOutput")
    if debug_taps:
        qval_d = nc.dram_tensor("qval_dbg", [128, NP], F32, kind="ExternalOutput")
        idx_d = nc.dram_tensor("idx_dbg", [128, NP + 8], U32, kind="ExternalOutput")
        gath_d = nc.dram_tensor(
            "gath_dbg", [128, NP * CQ], F32, kind="ExternalOutput"
        )

    with tile.TileContext(nc) as tc:
        with (
            tc.tile_pool(name="const", bufs=1) as constp,
            tc.tile_pool(name="psum", bufs=4, space="PSUM") as psump,
            tc.tile_pool(name="small", bufs=1) as smallp,
        ):
            # startup loads: constants first, then the first halves of the
            # ref-0 + feats tensors (tiles 0-7 runnable early), then rest
            mask_sb = constp.tile([128, WIN], BF16, tag="mask")
            ident_sb = constp.tile([128, 128], BF16, tag="ident")
            fth_sb = constp.tile([C, NT * 128], F16, tag="fth")
            ftl_sb = constp.tile([C, NT * 128], F16, tag="ftl")
            frhA_sb, frlA_sb, frhB_sb, frlB_sb = [], [], [], []
            for r in range(NREF):
                ha_ = constp.tile([C, WY * XA], F16, tag=f"frhA{r}")
                la_ = constp.tile([C, WY * XA], F16, tag=f"frlA{r}")
                hb_ = constp.tile([C, WY * XB2], F16, tag=f"frhB{r}")
                lb_ = constp.tile([C, WY * XB2], F16, tag=f"frlB{r}")
                frhA_sb.append(ha_)
                frlA_sb.append(la_)
                frhB_sb.append(hb_)
                frlB_sb.append(lb_)
            # A blocks (all refs) before any B block: pairs of tiles 0-7
            # become runnable while the B halves stream in
            nc.sync.dma_start(out=mask_sb[:], in_=mask_d.ap())
            nc.sync.dma_start(out=ident_sb[:], in_=ident_d.ap())
            nc.sync.dma_start(out=fth_sb[:, 0:1024], in_=fth_d.ap()[:, 0:1024])
            nc.sync.dma_start(out=frhA_sb[0][:], in_=frhA_d.ap()[0])
            nc.sync.dma_start(out=ftl_sb[:, 0:1024], in_=ftl_d.ap()[:, 0:1024])
            nc.sync.dma_start(out=frlA_sb[0][:], in_=frlA_d.ap()[0])
            for r in range(1, NREF):
                nc.sync.dma_start(out=frhA_sb[r][:], in_=frhA_d.ap()[r])
                nc.sync.dma_start(out=frlA_sb[r][:], in_=frlA_d.ap()[r])
            nc.sync.dma_start(out=fth_sb[:, 1024:], in_=fth_d.ap()[:, 1024:])
            nc.sync.dma_start(out=ftl_sb[:, 1024:], in_=ftl_d.ap()[:, 1024:])
            for r in range(NREF):
                nc.sync.dma_start(out=frhB_sb[r][:], in_=frhB_d.ap()[r])
                nc.sync.dma_start(out=frlB_sb[r][:], in_=frlB_d.ap()[r])

            # warm the PE (HAM un-throttles after ~3.4us of activity) with
            # dummy matmuls on the already-loaded mask/ident while the
            # feature tensors stream in
            warm_ps = psump.tile([128, 1024], F32, tag="ps")
            for _ in range(12):
                nc.tensor.matmul(
                    warm_ps[:, 0:512], ident_sb[:], mask_sb[:, 0:512],
                    start=True, stop=True,
                )

            # corr stream: 2 rotating regions of 8 pairs x 560 fp32
            corr_sb = smallp.tile([128, 2, BP, WIN], F32, tag="corr")
            qval = smallp.tile([128, NP], F32, tag="qval")
            # one private 8-wide window per find group (no WAR between
            # groups' finds and gathers)
            idx = smallp.tile([128, 8 * len(FIND_GROUPS)], U32, tag="idx")
            gath = smallp.tile([128, NP * CQ], F32, tag="gath")
            # qval is read in 8-wide in_max windows before all slots are
            # written; zero-init so the garbage needles are benign
            nc.gpsimd.memset(qval[:], 0.0)

            grp_of = {}
            for g, grp in enumerate(FIND_GROUPS):
                for p in grp:
                    grp_of[p] = g

            for t in range(NT):
                lhs_hi = fth_sb[:, t * 128 : (t + 1) * 128]
                lhs_lo = ftl_sb[:, t * 128 : (t + 1) * 128]
                for r in range(NREF):
                    p = t * NREF + r
                    b, j = p // BP, p % BP
                    if t < NT // 2:
                        hv = frhA_sb[r][:].rearrange("c (y x) -> c y x", x=XA)
                        lv = frlA_sb[r][:].rearrange("c (y x) -> c y x", x=XA)
                        x0 = t * XB
                    else:
                        hv = frhB_sb[r][:].rearrange("c (y x) -> c y x", x=XB2)
                        lv = frlB_sb[r][:].rearrange("c (y x) -> c y x", x=XB2)
                        x0 = t * XB - XB0
                    rh1 = hv[:, 0:HALF, x0 : x0 + WX]
                    rh2 = hv[:, HALF:WY, x0 : x0 + WX]
                    rl1 = lv[:, 0:HALF, x0 : x0 + WX]
                    rl2 = lv[:, HALF:WY, x0 : x0 + WX]
                    ps = psump.tile([128, 1024], F32, tag="ps")
                    ps1 = ps[:, 0:NHALF]
                    ps2 = ps[:, 512 : 512 + NHALF]
                    # 3-pass fp16 split ordered for stationary reuse
                    nc.tensor.matmul(ps1, lhs_hi, rh1, start=True, stop=False)
                    nc.tensor.matmul(ps2, lhs_hi, rh2, start=True, stop=False)
                    nc.tensor.matmul(ps1, lhs_hi, rl1, start=False, stop=False)
                    nc.tensor.matmul(ps2, lhs_hi, rl2, start=False, stop=False)
                    nc.tensor.matmul(ps1, lhs_lo, rh1, start=False, stop=False)
                    nc.tensor.matmul(ps2, lhs_lo, rh2, start=False, stop=False)
                    # valid-window mask (-1e30 outside own 13x13 patch)
                    nc.tensor.matmul(
                        ps1, ident_sb[:], mask_sb[:, 0:NHALF],
                        start=False, stop=True,
                    )
                    nc.tensor.matmul(
                        ps2, ident_sb[:], mask_sb[:, NHALF:WIN],
                        start=False, stop=True,
                    )
                    # ACT drains PSUM into the batched SBUF stream
                    psv = ps[:].rearrange("p (b n) -> p b n", b=2)[:, :, 0:NHALF]
                    nc.scalar.copy(
                        out=corr_sb[:, b % 2, j].rearrange(
                            "p (b n) -> p b n", b=2
                        ),
                        in_=psv,
                    )
                    g = grp_of[p]
                    grp = FIND_GROUPS[g]
                    if p == grp[-1]:
                        g0, gl = grp[0], len(grp)
                        j0 = g0 % BP
                        reg = corr_sb[:, b % 2, j0 : j0 + gl]
                        # per-pair max over the group's pairs
                        nc.vector.tensor_reduce(
                            out=qval[:, g0 : g0 + gl],
                            in_=reg,
                            axis=mybir.AxisListType.X,
                            op=mybir.AluOpType.max,
                        )
                        # argmax: one find_index8 over the group stream;
                        # match slots = the pair maxes (+ benign padding;
                        # qval window clamped in bounds; pair p's index
                        # lands in the group's private slot p - w0)
                        w0 = min(g0, NP - 8)
                        nc.vector.max_index(
                            out=idx[:, 8 * g : 8 * g + 8],
                            in_max=qval[:, w0 : w0 + 8],
                            in_values=reg.rearrange("p a b -> p (a b)"),
                        )
                        # gather each pair's argmax pixel (3 channels);
                        # stream positions are direct table row indices
                        for p2 in grp:
                            sl = 8 * g + (p2 - w0)
                            nc.gpsimd.indirect_dma_start(
                                out=gath[:, p2 * CQ : (p2 + 1) * CQ],
                                out_offset=None,
                                in_=qrp_d[g].ap(),
                                in_offset=IndirectOffsetOnAxis(
                                    ap=idx[:, sl : sl + 1], axis=0
                                ),
                            )

            if debug_taps:
                nc.sync.dma_start(out=qval_d.ap(), in_=qval[:])
                nc.sync.dma_start(out=idx_d.ap(), in_=idx[:])
                nc.sync.dma_start(out=gath_d.ap(), in_=gath[:])

            _softmax_combine(nc, smallp, qval, gath, out_d)

    nc.compile()
    return nc


def _softmax_combine(nc, smallp, qval, gath, out_d):
    # qval [128, (t, r)]; gath [128, (t, r, c)]
    qvv = qval[:].rearrange("p (t r) -> p t r", r=NREF)
    qv = [qvv[:, :, r] for r in range(NREF)]
    m01 = smallp.tile([128, NT], F32, tag="m01")
    nc.vector.tensor_tensor(
        out=m01[:], in0=qv[0], in1=qv[1], op=mybir.AluOpType.max
    )
    mm = smallp.tile([128, NT], F32, tag="mm")
    nc.vector.tensor_tensor(
        out=mm[:], in0=m01[:], in1=qv[2], op=mybir.AluOpType.max
    )
    es = []
    for r in range(NREF):
        e_ = smallp.tile([128, NT], F32, tag=f"e{r}")
        nc.vector.tensor_tensor(
            out=e_[:], in0=qv[r], in1=mm[:], op=mybir.AluOpType.subtract
        )
        nc.scalar.activation(
            out=e_[:], in_=e_[:], func=mybir.ActivationFunctionType.Exp
        )
        es.append(e_)
    ssum = smallp.tile([128, NT], F32, tag="ssum")
    nc.vector.tensor_tensor(
        out=ssum[:], in0=es[0][:], in1=es[1][:], op=mybir.AluOpType.add
    )
    nc.vector.tensor_tensor(
        out=ssum[:], in0=ssum[:], in1=es[2][:], op=mybir.AluOpType.add
    )
    rec = smallp.tile([128, NT], F32, tag="rec")
    nc.vector.reciprocal(out=rec[:], in_=ssum[:])

    gv = gath[:].rearrange("p (t r c) -> p t r c", r=NREF, c=CQ)
    oacc = smallp.tile([128, NT * CQ], F32, tag="oacc")
    oaccv = oacc[:].rearrange("p (s c) -> p s c", c=CQ)
    for r in range(NREF):
        w_ = smallp.tile([128, NT], F32, tag=f"w{r}")
        nc.vector.tensor_tensor(
            out=w_[:], in0=es[r][:], in1=rec[:], op=mybir.AluOpType.mult
        )
        wb = w_[:].rearrange("p (s o) -> p s o", o=1).to_broadcast([128, NT, CQ])
        if r == 0:
            nc.vector.tensor_tensor(
                out=oaccv, in0=gv[:, :, r], in1=wb, op=mybir.AluOpType.mult
            )
        else:
            term = smallp.tile([128, NT * CQ], F32, tag=f"term{r}")
            termv = term[:].rearrange("p (s c) -> p s c", c=CQ)
            nc.vector.tensor_tensor(
                out=termv, in0=gv[:, :, r], in1=wb, op=mybir.AluOpType.mult
            )
            nc.vector.tensor_tensor(
                out=oaccv, in0=oaccv, in1=termv, op=mybir.AluOpType.add
            )

    nc.sync.dma_start(out=out_d.ap(), in_=oacc[:])


def _host_prep(feats_r, feats_t, quantized_r):
    """Build the 8 per-core input maps."""
    import ml_dtypes

    frp_full = np.zeros((NREF, C, H + 2 * RAD, PW), np.float32)
    frp_full[:, :, RAD : RAD + H, RAD : RAD + W] = feats_r[:, 0]
    frh_full = frp_full.astype(np.float16)
    frl_full = (frp_full - frh_full.astype(np.float32)).astype(np.float16)

    def blocks(a, y0):
        # [C, WY, PW] band -> contiguous x-blocks A [C, WY*XA], B [C, WY*XB2]
        band = a[:, :, y0 : y0 + WY, :]
        A = np.ascontiguousarray(band[..., 0:XA]).reshape(NREF, C, WY * XA)
        B = np.ascontiguousarray(band[..., XB0:PW]).reshape(NREF, C, WY * XB2)
        return A, B

    ft = feats_t[0]
    fth = ft.astype(np.float16)
    ftl = (ft - fth.astype(np.float32)).astype(np.float16)

    qr = np.ascontiguousarray(quantized_r[:, 0, :, ::SUB, ::SUB], np.float32)
    qrp_full = np.zeros((NREF, H + 2 * RAD, PW, CQ), np.float32)
    qrp_full[:, RAD : RAD + H, RAD : RAD + W, :] = qr.transpose(0, 2, 3, 1)

    # mask[p=(yl,xl), n=(y',x')] = 0 inside pixel (yl,xl)'s own 13x13 patch
    yl = np.arange(ROWS)[:, None, None, None]
    xl = np.arange(XB)[None, :, None, None]
    yw = np.arange(WY)[None, None, :, None]
    xw = np.arange(WX)[None, None, None, :]
    valid = (
        (yw - yl >= 0) & (yw - yl < PS) & (xw - xl >= 0) & (xw - xl < PS)
    )
    mask = np.where(valid, 0.0, NEG).astype(ml_dtypes.bfloat16).reshape(128, WIN)
    ident = np.eye(128, dtype=np.float32).astype(ml_dtypes.bfloat16)

    def ft_layout(a):
        # [c, yl, t, xl] -> [c, t, yl, xl]: tile-major, pixels contiguous
        return np.ascontiguousarray(
            a.reshape(C, ROWS, NT, XB)
            .transpose(0, 2, 1, 3)
            .reshape(C, ROWS * W)
        )

    in_maps = []
    for k in range(NCORES):
        y0 = ROWS * k
        frhA, frhB = blocks(frh_full, y0)
        frlA, frlB = blocks(frl_full, y0)
        m = {
            "fth": ft_layout(fth[:, y0 : y0 + ROWS, :]),
            "ftl": ft_layout(ftl[:, y0 : y0 + ROWS, :]),
            "frhA": frhA,
            "frhB": frhB,
            "frlA": frlA,
            "frlB": frlB,
            "mask": mask,
            "ident": ident,
        }
        # per-find-group gather tables [len*WIN, CQ]; pair p = t*NREF + r
        qc = qrp_full[:, y0 : y0 + WY, :, :]  # [NREF, WY, PW, CQ]
        for g, grp in enumerate(FIND_GROUPS):
            tbl = np.empty((len(grp), WIN, CQ), np.float32)
            for jl, p in enumerate(grp):
                t, r = p // NREF, p % NREF
                tbl[jl] = qc[r, :, t * XB : t * XB + WX, :].reshape(WIN, CQ)
            m[f"qrp{g}"] = tbl.reshape(len(grp) * WIN, CQ)
        in_maps.append(m)
    return in_maps


def _install_ntff_shim():
    """This container's antenv lacks axon_hooks, so run_bass_kernel_spmd's
    trace path can't find the NTFF profile hook. Inject the module and
    register the ctypes-based hook from the boot script. Best-effort."""
    try:
        import sys
        import types

        if "antenv.axon_hooks" in sys.modules:
            return
        mod = types.ModuleType("antenv.axon_hooks")
        holder = [None]
        mod.set_axon_ntff_profile_hook = lambda h: holder.__setitem__(0, h)
        mod.get_axon_ntff_profile_hook = lambda: holder[0]
        sys.modules["antenv.axon_hooks"] = mod
        import antenv

        antenv.axon_hooks = mod
        from trn_agent_boot.trn_boot import _ntff_profile_via_ctypes

        hook = _ntff_profile_via_ctypes("/opt/axon/libaxon_pjrt.so")
        if hook is not None:
            mod.set_axon_ntff_profile_hook(hook)
    except Exception as e:  # pragma: no cover - tracing is best-effort
        print(f"ntff shim install failed: {e}")


last_exec_time_ns = None


def kernel(feats_r, feats_t, quantized_r, ref_index=None, current_ind=None):
    global last_exec_time_ns
    feats_r = np.asarray(feats_r, np.float32)
    feats_t = np.asarray(feats_t, np.float32)
    quantized_r = np.asarray(quantized_r, np.float32)

    in_maps = _host_prep(feats_r, feats_t, quantized_r)

    if "nc" not in _CACHE:
        _CACHE["nc"] = _build_program()
    nc = _CACHE["nc"]

    trace = bool(int(os.environ.get("KERNEL_TRACE", "0")))
    kwargs = {}
    if trace:
        _install_ntff_shim()
        tdir = os.environ.get("KERNEL_TRACE_DIR")
        if tdir:
            os.makedirs(tdir, exist_ok=True)
            kwargs["tmpdir"] = tdir
    res = run_bass_kernel_spmd(
        nc, in_maps, list(range(NCORES)), trace=trace, **kwargs
    )
    last_exec_time_ns = res.exec_time_ns

    out = np.concatenate(
        [_unshard_core(res.results[k]["out"]) for k in range(NCORES)], axis=1
    )
    return np.ascontiguousarray(out.reshape(1, CQ, H, W), np.float32)


def _unshard_core(raw):
    # raw [128, NT*CQ] with partition p=(yl,xl), free (t, c) -> [CQ, ROWS, W]
    r = np.asarray(raw).reshape(ROWS, XB, NT, CQ)
    return r.transpose(3, 0, 2, 1).reshape(CQ, ROWS, W)


# revision 48
# speedup vs baseline: 1.0959x; 1.0959x over previous
"""Trainium2 Bass kernel for nn_Colorizer (retrieval_knn).

Computation (per reference frame r of 3, for each pixel p of a 128x128 image):
  corr[r, n, p] = <feats_t[:, p], feats_r[r, :, p + offset(n)]>   n in 13x13 window
  q_val[r, p]  = max_n corr ; q_idx[r, p] = argmax_n corr (first occurrence)
  gathered[r, c, p] = quantized_sub[r, c, p + offset(q_idx)]      (zero padded)
  out[c, p] = sum_r softmax_r(q_val)[r] * gathered[r, c, p]

Sharding: the spatial h dim is split into 8 bands of 16 rows (one per core);
each core handles all 3 refs for its band, so the softmax over refs is local
and no device collective is needed.  Host reassembles the row bands.

Device algorithm per core, per (tile, ref) pair (16 tiles of 16x8 pixels,
3 refs -> 48 pairs):
  * TensorE computes the Gram matrix between the tile's feats_t vectors and
    the 28x20 zero-padded feats_r halo window (560 columns over 2 PSUM banks)
    as a 3-pass fp16 split (hi*hi + hi*lo + lo*hi; the dropped lo*lo term is
    ~6e-6 which is far below the 3.3e-5 min top-2 corr gap on these inputs,
    so argmax and softmax match fp32), then accumulates a -1e30 valid-window
    mask via a bf16 identity matmul.  A dozen dummy matmuls on the
    already-loaded mask warm the PE's HAM clock gate during input load.
  * ScalarE (ACT) copies each masked PSUM pair to a contiguous SBUF stream,
    releasing PSUM for the next matmuls.
  * VectorE computes per-pair maxes (tensor_reduce, batched per find group)
    and argmaxes (one find_index8 per group: the group's pair maxes are the
    8 match slots, the group's concatenated corr segments are the stream, so
    the found stream position is directly the gather-table row).  Group
    sizes ramp 1,1,2,4,8... so the serial gpsimd gather stream starts as
    early as possible.
  * GpSimd issues one indirect DMA per pair ([128,1] offsets - the SWDGE
    only honors one offset per partition) gathering the argmax pixel
    (3 channels) from the group's DRAM table.
  * A tiny fp32 softmax over the 3 refs weights the gathered values.

Engine budget per core (measured): PE ~50us (warm ~283ns/280-col matmul),
DVE ~63us (the 2 scans/pair at ~1 elem/cycle are the floor), ScalarE ~36us,
GpSimd gathers 48 x ~1.4us (descriptor generation on the Q7 cores; this
serial stream plus the DVE stream set the ~100us span), ~14us of fixed
NEFF preamble/exit-drain overhead.
"""

import os

import numpy as np

import concourse.bass as bass
import concourse.mybir as mybir
import concourse.tile as tile
from concourse import bacc
from concourse.bass import IndirectOffsetOnAxis
from concourse.bass_utils import run_bass_kernel_spmd

F32 = mybir.dt.float32
F16 = mybir.dt.float16
BF16 = mybir.dt.bfloat16
U32 = mybir.dt.uint32

NCORES = 8
NREF, C, H, W = 3, 128, 128, 128
RAD = 6                      # patch radius
PS = 2 * RAD + 1             # 13
CQ = 3                       # quantized channels
SUB = 4                      # quantized_r spatial subsample stride

ROWS = H // NCORES           # 16 rows per core
XB = 8                       # x block size
NT = W // XB                 # 16 tiles per ref
WY = ROWS + 2 * RAD          # 28 window rows
WX = XB + 2 * RAD            # 20 window cols
WIN = WY * WX                # 560
HALF = WY // 2               # 14 window rows per PSUM bank
NHALF = HALF * WX            # 280 columns per matmul
PW = W + 2 * RAD             # 140 padded width
XA = 76                      # x-block A: padded cols [0, 76), tiles 0-7
XB2 = 84                     # x-block B: padded cols [56, 140), tiles 8-15
XB0 = PW - XB2               # 56, B block origin
NP = NREF * NT               # 48 (tile, ref) pairs
NB = 6                       # corr-buffer batches
BP = NP // NB                # 8 pairs per batch
# find groups: small at the start (begin the serial gpsimd gather stream
# early) and at the end (shorten the post-last-find gather tail),
# 8-wide in the middle (less DVE overhead)
_GRP_SIZES = [1, 1, 2, 4, 4, 4, 4, 4, 4, 4, 4, 4, 4, 4]
FIND_GROUPS = []
_s = 0
for _n in _GRP_SIZES:
    FIND_GROUPS.append(list(range(_s, _s + _n)))
    _s += _n
NEG = -1.0e30

_CACHE: dict = {}


def _build_program(debug_taps: bool = False) -> bacc.Bacc:
    nc = bacc.Bacc("TRN2", target_bir_lowering=False, debug=False)

    fth_d = nc.dram_tensor("fth", [C, NT * 128], F16, kind="ExternalInput")
    ftl_d = nc.dram_tensor("ftl", [C, NT * 128], F16, kind="ExternalInput")
    # fr windows pre-split into two overlapping contiguous x-blocks:
    # A = padded cols [0, XA) (tiles 0-7), B = [PW-XB2, PW) (tiles 8-15)
    frhA_d = nc.dram_tensor("frhA", [NREF, C, WY * XA], F16, kind="ExternalInput")
    frlA_d = nc.dram_tensor("frlA", [NREF, C, WY * XA], F16, kind="ExternalInput")
    frhB_d = nc.dram_tensor("frhB", [NREF, C, WY * XB2], F16, kind="ExternalInput")
    frlB_d = nc.dram_tensor("frlB", [NREF, C, WY * XB2], F16, kind="ExternalInput")
    mask_d = nc.dram_tensor("mask", [128, WIN], BF16, kind="ExternalInput")
    ident_d = nc.dram_tensor("ident", [128, 128], BF16, kind="ExternalInput")
    # per-find-group gather tables: row j*WIN + n = quantized pixel (3ch)
    # of window position n for the group's j-th pair (find_index8 returns
    # stream positions, which are then direct row indices)
    qrp_d = [
        nc.dram_tensor(f"qrp{g}", [len(grp) * WIN, CQ], F32, kind="ExternalInput")
        for g, grp in enumerate(FIND_GROUPS)
    ]
    # raw layout [pixel_partition=(yl,xl), tile, channel]; host untangles
    out_d = nc.dram_tensor("out", [128, NT * CQ], F32, kind="ExternalOutput")
    if debug_taps:
        qval_d = nc.dram_tensor("qval_dbg", [128, NP], F32, kind="ExternalOutput")
        idx_d = nc.dram_tensor("idx_dbg", [128, NP + 8], U32, kind="ExternalOutput")
        gath_d = nc.dram_tensor(
            "gath_dbg", [128, NP * CQ], F32, kind="ExternalOutput"
        )

    with tile.TileContext(nc) as tc:
        with (
            tc.tile_pool(name="const", bufs=1) as constp,
            tc.tile_pool(name="psum", bufs=4, space="PSUM") as psump,
            tc.tile_pool(name="small", bufs=1) as smallp,
        ):
            # startup loads: constants first, then the first halves of the
            # ref-0 + feats tensors (tiles 0-7 runnable early), then rest
            mask_sb = constp.tile([128, WIN], BF16, tag="mask")
            ident_sb = constp.tile([128, 128], BF16, tag="ident")
            fth_sb = constp.tile([C, NT * 128], F16, tag="fth")
            ftl_sb = constp.tile([C, NT * 128], F16, tag="ftl")
            frhA_sb, frlA_sb, frhB_sb, frlB_sb = [], [], [], []
            for r in range(NREF):
                ha_ = constp.tile([C, WY * XA], F16, tag=f"frhA{r}")
                la_ = constp.tile([C, WY * XA], F16, tag=f"frlA{r}")
                hb_ = constp.tile([C, WY * XB2], F16, tag=f"frhB{r}")
                lb_ = constp.tile([C, WY * XB2], F16, tag=f"frlB{r}")
                frhA_sb.append(ha_)
                frlA_sb.append(la_)
                frhB_sb.append(hb_)
                frlB_sb.append(lb_)
            # A blocks (all refs) before any B block: pairs of tiles 0-7
            # become runnable while the B halves stream in
            nc.sync.dma_start(out=mask_sb[:], in_=mask_d.ap())
            nc.sync.dma_start(out=ident_sb[:], in_=ident_d.ap())
            nc.sync.dma_start(out=fth_sb[:, 0:1024], in_=fth_d.ap()[:, 0:1024])
            nc.sync.dma_start(out=frhA_sb[0][:], in_=frhA_d.ap()[0])
            nc.sync.dma_start(out=ftl_sb[:, 0:1024], in_=ftl_d.ap()[:, 0:1024])
            nc.sync.dma_start(out=frlA_sb[0][:], in_=frlA_d.ap()[0])
            for r in range(1, NREF):
                nc.sync.dma_start(out=frhA_sb[r][:], in_=frhA_d.ap()[r])
                nc.sync.dma_start(out=frlA_sb[r][:], in_=frlA_d.ap()[r])
            nc.sync.dma_start(out=fth_sb[:, 1024:], in_=fth_d.ap()[:, 1024:])
            nc.sync.dma_start(out=ftl_sb[:, 1024:], in_=ftl_d.ap()[:, 1024:])
            for r in range(NREF):
                nc.sync.dma_start(out=frhB_sb[r][:], in_=frhB_d.ap()[r])
                nc.sync.dma_start(out=frlB_sb[r][:], in_=frlB_d.ap()[r])

            # warm the PE (HAM un-throttles after ~3.4us of activity) with
            # dummy matmuls on the already-loaded mask/ident while the
            # feature tensors stream in
            warm_ps = psump.tile([128, 1024], F32, tag="ps")
            for _ in range(12):
                nc.tensor.matmul(
                    warm_ps[:, 0:512], ident_sb[:], mask_sb[:, 0:512],
                    start=True, stop=True,
                )

            # corr stream: 2 rotating regions of 8 pairs x 560 fp32
            corr_sb = smallp.tile([128, 2, BP, WIN], F32, tag="corr")
            qval = smallp.tile([128, NP], F32, tag="qval")
            # one private 8-wide window per find group (no WAR between
            # groups' finds and gathers)
            idx = smallp.tile([128, 8 * len(FIND_GROUPS)], U32, tag="idx")
            gath = smallp.tile([128, NP * CQ], F32, tag="gath")
            # qval is read in 8-wide in_max windows before all slots are
            # written; zero-init so the garbage needles are benign
            nc.gpsimd.memset(qval[:], 0.0)

            grp_of = {}
            for g, grp in enumerate(FIND_GROUPS):
                for p in grp:
                    grp_of[p] = g

            for t in range(NT):
                lhs_hi = fth_sb[:, t * 128 : (t + 1) * 128]
                lhs_lo = ftl_sb[:, t * 128 : (t + 1) * 128]
                for r in range(NREF):
                    p = t * NREF + r
                    b, j = p // BP, p % BP
                    if t < NT // 2:
                        hv = frhA_sb[r][:].rearrange("c (y x) -> c y x", x=XA)
                        lv = frlA_sb[r][:].rearrange("c (y x) -> c y x", x=XA)
                        x0 = t * XB
                    else:
                        hv = frhB_sb[r][:].rearrange("c (y x) -> c y x", x=XB2)
                        lv = frlB_sb[r][:].rearrange("c (y x) -> c y x", x=XB2)
                        x0 = t * XB - XB0
                    rh1 = hv[:, 0:HALF, x0 : x0 + WX]
                    rh2 = hv[:, HALF:WY, x0 : x0 + WX]
                    rl1 = lv[:, 0:HALF, x0 : x0 + WX]
                    rl2 = lv[:, HALF:WY, x0 : x0 + WX]
                    ps = psump.tile([128, 1024], F32, tag="ps")
                    ps1 = ps[:, 0:NHALF]
                    ps2 = ps[:, 512 : 512 + NHALF]
                    # 3-pass fp16 split ordered for stationary reuse
                    nc.tensor.matmul(ps1, lhs_hi, rh1, start=True, stop=False)
                    nc.tensor.matmul(ps2, lhs_hi, rh2, start=True, stop=False)
                    nc.tensor.matmul(ps1, lhs_hi, rl1, start=False, stop=False)
                    nc.tensor.matmul(ps2, lhs_hi, rl2, start=False, stop=False)
                    nc.tensor.matmul(ps1, lhs_lo, rh1, start=False, stop=False)
                    nc.tensor.matmul(ps2, lhs_lo, rh2, start=False, stop=False)
                    # valid-window mask (-1e30 outside own 13x13 patch)
                    nc.tensor.matmul(
                        ps1, ident_sb[:], mask_sb[:, 0:NHALF],
                        start=False, stop=True,
                    )
                    nc.tensor.matmul(
                        ps2, ident_sb[:], mask_sb[:, NHALF:WIN],
                        start=False, stop=True,
                    )
                    # ACT drains PSUM into the batched SBUF stream
                    psv = ps[:].rearrange("p (b n) -> p b n", b=2)[:, :, 0:NHALF]
                    nc.scalar.copy(
                        out=corr_sb[:, b % 2, j].rearrange(
                            "p (b n) -> p b n", b=2
                        ),
                        in_=psv,
                    )
                    g = grp_of[p]
                    grp = FIND_GROUPS[g]
                    if p == grp[-1]:
                        g0, gl = grp[0], len(grp)
                        j0 = g0 % BP
                        reg = corr_sb[:, b % 2, j0 : j0 + gl]
                        # per-pair max over the group's pairs
                        nc.vector.tensor_reduce(
                            out=qval[:, g0 : g0 + gl],
                            in_=reg,
                            axis=mybir.AxisListType.X,
                            op=mybir.AluOpType.max,
                        )
                        # argmax: one find_index8 over the group stream;
                        # match slots = the pair maxes (+ benign padding;
                        # qval window clamped in bounds; pair p's index
                        # lands in the group's private slot p - w0)
                        w0 = min(g0, NP - 8)
                        nc.vector.max_index(
                            out=idx[:, 8 * g : 8 * g + 8],
                            in_max=qval[:, w0 : w0 + 8],
                            in_values=reg.rearrange("p a b -> p (a b)"),
                        )
                        # gather each pair's argmax pixel (3 channels);
                        # stream positions are direct table row indices
                        for p2 in grp:
                            sl = 8 * g + (p2 - w0)
                            nc.gpsimd.indirect_dma_start(
                                out=gath[:, p2 * CQ : (p2 + 1) * CQ],
                                out_offset=None,
                                in_=qrp_d[g].ap(),
                                in_offset=IndirectOffsetOnAxis(
                                    ap=idx[:, sl : sl + 1], axis=0
                                ),
                            )

            if debug_taps:
                nc.sync.dma_start(out=qval_d.ap(), in_=qval[:])
                nc.sync.dma_start(out=idx_d.ap(), in_=idx[:])
                nc.sync.dma_start(out=gath_d.ap(), in_=gath[:])

            _softmax_combine(nc, smallp, qval, gath, out_d)

    nc.compile()
    return nc


def _softmax_combine(nc, smallp, qval, gath, out_d):
    # qval [128, (t, r)]; gath [128, (t, r, c)]
    qvv = qval[:].rearrange("p (t r) -> p t r", r=NREF)
    qv = [qvv[:, :, r] for r in range(NREF)]
    m01 = smallp.tile([128, NT], F32, tag="m01")
    nc.vector.tensor_tensor(
        out=m01[:], in0=qv[0], in1=qv[1], op=mybir.AluOpType.max
    )
    mm = smallp.tile([128, NT], F32, tag="mm")
    nc.vector.tensor_tensor(
        out=mm[:], in0=m01[:], in1=qv[2], op=mybir.AluOpType.max
    )
    es = []
    for r in range(NREF):
        e_ = smallp.tile([128, NT], F32, tag=f"e{r}")
        nc.vector.tensor_tensor(
            out=e_[:], in0=qv[r], in1=mm[:], op=mybir.AluOpType.subtract
        )
        nc.scalar.activation(
            out=e_[:], in_=e_[:], func=mybir.ActivationFunctionType.Exp
        )
        es.append(e_)
    ssum = smallp.tile([128, NT], F32, tag="ssum")
    nc.vector.tensor_tensor(
        out=ssum[:], in0=es[0][:], in1=es[1][:], op=mybir.AluOpType.add
    )
    nc.vector.tensor_tensor(
        out=ssum[:], in0=ssum[:], in1=es[2][:], op=mybir.AluOpType.add
    )
    rec = smallp.tile([128, NT], F32, tag="rec")
    nc.vector.reciprocal(out=rec[:], in_=ssum[:])

    gv = gath[:].rearrange("p (t r c) -> p t r c", r=NREF, c=CQ)
    oacc = smallp.tile([128, NT * CQ], F32, tag="oacc")
    oaccv = oacc[:].rearrange("p (s c) -> p s c", c=CQ)
    for r in range(NREF):
        w_ = smallp.tile([128, NT], F32, tag=f"w{r}")
        nc.vector.tensor_tensor(
            out=w_[:], in0=es[r][:], in1=rec[:], op=mybir.AluOpType.mult
        )
        wb = w_[:].rearrange("p (s o) -> p s o", o=1).to_broadcast([128, NT, CQ])
        if r == 0:
            nc.vector.tensor_tensor(
                out=oaccv, in0=gv[:, :, r], in1=wb, op=mybir.AluOpType.mult
            )
        else:
            term = smallp.tile([128, NT * CQ], F32, tag=f"term{r}")
            termv = term[:].rearrange("p (s c) -> p s c", c=CQ)
            nc.vector.tensor_tensor(
                out=termv, in0=gv[:, :, r], in1=wb, op=mybir.AluOpType.mult
            )
            nc.vector.tensor_tensor(
                out=oaccv, in0=oaccv, in1=termv, op=mybir.AluOpType.add
            )

    nc.sync.dma_start(out=out_d.ap(), in_=oacc[:])


def _host_prep(feats_r, feats_t, quantized_r):
    """Build the 8 per-core input maps."""
    import ml_dtypes

    frp_full = np.zeros((NREF, C, H + 2 * RAD, PW), np.float32)
    frp_full[:, :, RAD : RAD + H, RAD : RAD + W] = feats_r[:, 0]
    frh_full = frp_full.astype(np.float16)
    frl_full = (frp_full - frh_full.astype(np.float32)).astype(np.float16)

    def blocks(a, y0):
        # [C, WY, PW] band -> contiguous x-blocks A [C, WY*XA], B [C, WY*XB2]
        band = a[:, :, y0 : y0 + WY, :]
        A = np.ascontiguousarray(band[..., 0:XA]).reshape(NREF, C, WY * XA)
        B = np.ascontiguousarray(band[..., XB0:PW]).reshape(NREF, C, WY * XB2)
        return A, B

    ft = feats_t[0]
    fth = ft.astype(np.float16)
    ftl = (ft - fth.astype(np.float32)).astype(np.float16)

    qr = np.ascontiguousarray(quantized_r[:, 0, :, ::SUB, ::SUB], np.float32)
    qrp_full = np.zeros((NREF, H + 2 * RAD, PW, CQ), np.float32)
    qrp_full[:, RAD : RAD + H, RAD : RAD + W, :] = qr.transpose(0, 2, 3, 1)

    # mask[p=(yl,xl), n=(y',x')] = 0 inside pixel (yl,xl)'s own 13x13 patch
    yl = np.arange(ROWS)[:, None, None, None]
    xl = np.arange(XB)[None, :, None, None]
    yw = np.arange(WY)[None, None, :, None]
    xw = np.arange(WX)[None, None, None, :]
    valid = (
        (yw - yl >= 0) & (yw - yl < PS) & (xw - xl >= 0) & (xw - xl < PS)
    )
    mask = np.where(valid, 0.0, NEG).astype(ml_dtypes.bfloat16).reshape(128, WIN)
    ident = np.eye(128, dtype=np.float32).astype(ml_dtypes.bfloat16)

    def ft_layout(a):
        # [c, yl, t, xl] -> [c, t, yl, xl]: tile-major, pixels contiguous
        return np.ascontiguousarray(
            a.reshape(C, ROWS, NT, XB)
            .transpose(0, 2, 1, 3)
            .reshape(C, ROWS * W)
        )

    in_maps = []
    for k in range(NCORES):
        y0 = ROWS * k
        frhA, frhB = blocks(frh_full, y0)
        frlA, frlB = blocks(frl_full, y0)
        m = {
            "fth": ft_layout(fth[:, y0 : y0 + ROWS, :]),
            "ftl": ft_layout(ftl[:, y0 : y0 + ROWS, :]),
            "frhA": frhA,
            "frhB": frhB,
            "frlA": frlA,
            "frlB": frlB,
            "mask": mask,
            "ident": ident,
        }
        # per-find-group gather tables [len*WIN, CQ]; pair p = t*NREF + r
        qc = qrp_full[:, y0 : y0 + WY, :, :]  # [NREF, WY, PW, CQ]
        for g, grp in enumerate(FIND_GROUPS):
            tbl = np.empty((len(grp), WIN, CQ), np.float32)
            for jl, p in enumerate(grp):
                t, r = p // NREF, p % NREF
                tbl[jl] = qc[r, :, t * XB : t * XB + WX, :].reshape(WIN, CQ)
            m[f"qrp{g}"] = tbl.reshape(len(grp) * WIN, CQ)
        in_maps.append(m)
    return in_maps


def _install_ntff_shim():
    """This container's antenv lacks axon_hooks, so run_bass_kernel_spmd's
    trace path can't find the NTFF profile hook. Inject the module and
    register the ctypes-based hook from the boot script. Best-effort."""
    try:
        import sys
        import types

        if "antenv.axon_hooks" in sys.modules:
            return
        mod = types.ModuleType("antenv.axon_hooks")
        holder = [None]
        mod.set_axon_ntff_profile_hook = lambda h: holder.__setitem__(0, h)
        mod.get_axon_ntff_profile_hook = lambda: holder[0]
        sys.modules["antenv.axon_hooks"] = mod
        import antenv

        antenv.axon_hooks = mod
        from trn_agent_boot.trn_boot import _ntff_profile_via_ctypes

        hook = _ntff_profile_via_ctypes("/opt/axon/libaxon_pjrt.so")
        if hook is not None:
            mod.set_axon_ntff_profile_hook(hook)
    except Exception as e:  # pragma: no cover - tracing is best-effort
        print(f"ntff shim install failed: {e}")


last_exec_time_ns = None


def kernel(feats_r, feats_t, quantized_r, ref_index=None, current_ind=None):
    global last_exec_time_ns
    feats_r = np.asarray(feats_r, np.float32)
    feats_t = np.asarray(feats_t, np.float32)
    quantized_r = np.asarray(quantized_r, np.float32)

    in_maps = _host_prep(feats_r, feats_t, quantized_r)

    if "nc" not in _CACHE:
        _CACHE["nc"] = _build_program()
    nc = _CACHE["nc"]

    trace = bool(int(os.environ.get("KERNEL_TRACE", "0")))
    kwargs = {}
    if trace:
        _install_ntff_shim()
        tdir = os.environ.get("KERNEL_TRACE_DIR")
        if tdir:
            os.makedirs(tdir, exist_ok=True)
            kwargs["tmpdir"] = tdir
    res = run_bass_kernel_spmd(
        nc, in_maps, list(range(NCORES)), trace=trace, **kwargs
    )
    last_exec_time_ns = res.exec_time_ns

    out = np.concatenate(
        [_unshard_core(res.results[k]["out"]) for k in range(NCORES)], axis=1
    )
    return np.ascontiguousarray(out.reshape(1, CQ, H, W), np.float32)


def _unshard_core(raw):
    # raw [128, NT*CQ] with partition p=(yl,xl), free (t, c) -> [CQ, ROWS, W]
    r = np.asarray(raw).reshape(ROWS, XB, NT, CQ)
    return r.transpose(3, 0, 2, 1).reshape(CQ, ROWS, W)
